# revision 32
# baseline (speedup 1.0000x reference)
"""Distributed FlashRotarySelfAttention kernel for 8 TRN2 NeuronCores.

Reference computation (per nn_FlashRotarySelfAttention):
  qkv = x @ Wqkv;  k, q, v = split(qkv, 3)  [k first!]
  k, q = rope(k), rope(q)
  out = causal_softmax(q k^T / sqrt(Dh)) @ v
  return out @ Wproj

Sharding: tensor-parallel over heads for QKV+attention, row-block
parallel for the projection. Core i owns heads {2i, 2i+1}:
  - column-parallel Wqkv (k|q|v columns of its 2 heads)
  - attention fully local per (batch, head); the two head-groups of a
    (batch, q-chunk) pair are emitted interleaved, and the pairs are
    interleaved into the QKV s-chunk loop, so ACT/DVE softmax work
    hides under the PE-bound QKV matmuls and the PE stays warm
  - one small AllToAll per batch redistributes the attention output
    from head-sharded to row-block-sharded (256 rows/core/batch);
    batch 0's AllToAll overlaps batch 1's attention, and batch 0's
    projection matmuls are injected as fillers between the last
    attention pair's matmuls
  - each core then computes the FULL projection (all 2048 output
    channels) for its own 512 rows -> output is row-sharded, no
    further communication

All tensors are pre-cast to bf16 on the host; x is also pre-transposed
on the host, so the kernel does no on-chip casts or transposes.
Matmuls run in bf16 with fp32 PSUM accumulation. RoPE consumes one
ACT copy (PSUM fp32 -> SBUF bf16) + 4 DVE multiplies using a
sign-folded sin table. Softmax skips max-subtraction (scores are O(10)
here); the denominator is accumulated on DVE and reduced across
partitions by a ones-matmul, reciprocal on DVE.
"""

from contextlib import ExitStack

import numpy as np
import ml_dtypes

import concourse.bacc as bacc
import concourse.mybir as mybir
import concourse.tile as tile
from concourse.bass_utils import run_bass_kernel_spmd

# Problem shapes (hardcoded per contest rules).
B, S, C, H = 2, 2048, 2048, 16
Dh = C // H                      # 128
BS = B * S                       # 4096
N_CORES = 8
H_LOC = H // N_CORES             # 2 heads per core
ROWS = S // N_CORES              # 256 output rows per core per batch
ROPE_THETA = 10000.0
SCALE = float(Dh) ** -0.5

F32 = mybir.dt.float32
BF16 = mybir.dt.bfloat16

P = 128            # partitions
QCH = 512          # q-chunk (matmul free dim)
N_SC = BS // QCH   # 8 s-chunks over B*S
N_CC = C // P      # 16 contraction chunks
N_QC = S // QCH    # 4 q-chunks per batch
N_KT = S // P      # 16 k-tiles per batch
AV_LAG = 2         # per-group av lag; x2 effective depth with pair interleave


def _host_constants():
    """Input-independent tables computed on host (compile-time constants)."""
    half = Dh // 2
    inv_freq = 1.0 / (ROPE_THETA ** (np.arange(0, half, dtype=np.float64) / half))
    ang = np.arange(S, dtype=np.float64)[None, :] * inv_freq[:, None]   # [64, S]
    cos_t = np.tile(np.cos(ang), (2, 1)).astype(ml_dtypes.bfloat16)     # [128, S]
    # sign-folded sin, laid out so each RoPE multiply reads both inputs at
    # the same base partition: rows 0-63 (+sin) pair with tb[0:64] to make
    # out[64:128]; rows 64-127 (-sin) pair with tb[64:128] to make out[0:64]
    sin_t = np.concatenate([np.sin(ang), -np.sin(ang)], axis=0).astype(
        ml_dtypes.bfloat16
    )                                                                   # [128, S]
    # upper-triangular (incl diag) strip mask: keep iff q_local >= k_local
    kk = np.arange(P)[:, None]
    cc = np.arange(P)[None, :]
    tri = (cc >= kk).astype(ml_dtypes.bfloat16)                         # [128, 128]
    ones = np.ones((P, P), dtype=ml_dtypes.bfloat16)
    return cos_t, sin_t, tri, ones


def build_nc():
    nc = bacc.Bacc(None, num_devices=N_CORES)

    xt_in = nc.declare_dram_parameter("xT", [P, N_SC, N_CC, QCH], BF16, isOutput=False)
    wqkv_in = nc.declare_dram_parameter("wqkv", [P, 3, N_CC, 256], BF16, isOutput=False)
    wproj_in = nc.declare_dram_parameter("wproj", [P, N_CC, C], BF16, isOutput=False)
    cos_in = nc.declare_dram_parameter("cos_t", [P, S], BF16, isOutput=False)
    sin_in = nc.declare_dram_parameter("sin_t", [P, S], BF16, isOutput=False)
    tri_in = nc.declare_dram_parameter("tri", [P, P], BF16, isOutput=False)
    ones_in = nc.declare_dram_parameter("ones", [P, P], BF16, isOutput=False)
    out_ext = nc.declare_dram_parameter("out", [B * ROWS, C], F32, isOutput=True)

    with tile.TileContext(nc) as tc, ExitStack() as ctx:
        consts = ctx.enter_context(tc.tile_pool(name="consts", bufs=1))
        qkvp = ctx.enter_context(tc.tile_pool(name="qkvp", bufs=1))
        xt_pool = ctx.enter_context(tc.tile_pool(name="xt", bufs=2))
        rope_pool = ctx.enter_context(tc.tile_pool(name="rope", bufs=3))
        probs_pool = ctx.enter_context(tc.tile_pool(name="probs", bufs=4))
        acc_pool = ctx.enter_context(tc.tile_pool(name="accs", bufs=2))
        attn_pool = ctx.enter_context(tc.tile_pool(name="attn", bufs=3))
        gt_pool = ctx.enter_context(tc.tile_pool(name="gt", bufs=1))
        outp_pool = ctx.enter_context(tc.tile_pool(name="outp", bufs=2))
        dram = ctx.enter_context(tc.tile_pool(name="dram", bufs=1, space="DRAM"))
        mmps = ctx.enter_context(tc.tile_pool(name="mmps", bufs=2, space="PSUM"))
        sps_pool = ctx.enter_context(tc.tile_pool(name="sps", bufs=2, space="PSUM"))
        ops_pool = ctx.enter_context(tc.tile_pool(name="ops", bufs=2, space="PSUM"))

        # ---- Startup DMAs: k weights + first x half first -----------------
        wq_sb = consts.tile([P, 3, N_CC, 256], BF16)
        nc.gpsimd.dma_start(wq_sb[:, 0, :, :], wqkv_in[:, 0, :, :])

        xts = [None] * N_SC

        def load_xt(sc):
            xt = xt_pool.tile([P, N_CC, QCH], BF16, tag="xt", name=f"xt{sc}")
            nc.sync.dma_start(xt[:], xt_in[:, sc, :, :])
            return lambda cc: xt[:, cc, :]

        # chunk 0 in two halves so the first k-matmul chain can start after
        # just 1 MB of x has landed
        x0a = xt_pool.tile([P, N_CC // 2, QCH], BF16, tag="xt", name="x0a")
        nc.sync.dma_start(x0a[:], xt_in[:, 0, 0:N_CC // 2, :])
        x0b = xt_pool.tile([P, N_CC // 2, QCH], BF16, tag="xt", name="x0b")
        nc.sync.dma_start(x0b[:], xt_in[:, 0, N_CC // 2:, :])
        xts[0] = lambda cc: (x0a if cc < N_CC // 2 else x0b)[:, cc % (N_CC // 2), :]

        nc.gpsimd.dma_start(wq_sb[:, 1, :, :], wqkv_in[:, 1, :, :])
        nc.gpsimd.dma_start(wq_sb[:, 2, :, :], wqkv_in[:, 2, :, :])

        cos_sb = consts.tile([P, S], BF16)
        nc.scalar.dma_start(cos_sb[:], cos_in[:])
        sin_sb = consts.tile([P, S], BF16)
        nc.scalar.dma_start(sin_sb[:], sin_in[:])
        tri_sb = consts.tile([P, P], BF16)
        nc.scalar.dma_start(tri_sb[:], tri_in[:])
        ones_sb = consts.tile([P, P], BF16)
        nc.scalar.dma_start(ones_sb[:], ones_in[:])

        # wproj is loaded later (emitted after s-chunk 2) off the critical
        # startup HBM window
        wproj_sb = consts.tile([P, N_CC, C], BF16)

        # Resident activations: d-major q/k, k-major v. bh = hl*2 + b
        q_sb = qkvp.tile([P, 2 * H_LOC, S], BF16)
        k_sb = qkvp.tile([P, 2 * H_LOC, S], BF16)
        v_sb = qkvp.tile([P, B, N_KT, H_LOC * Dh], BF16)

        def qkv_chunk(sc, get_xt):
            b, s0 = divmod(sc, N_QC)
            s0 *= QCH                      # position offset within batch
            # k (part 0) and q (part 1): matmul -> RoPE -> bf16 resident
            for part in range(2):
                for hp in range(H_LOC):
                    ps = mmps.tile([P, QCH], F32, tag="mm")
                    for cci in range(N_CC):
                        nc.tensor.matmul(
                            ps[:],
                            lhsT=wq_sb[:, part, cci, hp * P:(hp + 1) * P],
                            rhs=get_xt(cci),
                            start=(cci == 0),
                            stop=(cci == N_CC - 1),
                        )
                    tb = rope_pool.tile([P, QCH], BF16, tag="rt")
                    nc.vector.tensor_copy(tb[:], ps[:])
                    m1 = rope_pool.tile([P, QCH], BF16, tag="rt")
                    m2 = rope_pool.tile([P, QCH], BF16, tag="rt")
                    nc.vector.tensor_tensor(
                        m1[:], tb[:], cos_sb[:, s0:s0 + QCH], mybir.AluOpType.mult
                    )
                    nc.vector.tensor_tensor(
                        m2[0:64, :], tb[64:128, :], sin_sb[64:128, s0:s0 + QCH],
                        mybir.AluOpType.mult,
                    )
                    nc.vector.tensor_tensor(
                        m2[64:128, :], tb[0:64, :], sin_sb[0:64, s0:s0 + QCH],
                        mybir.AluOpType.mult,
                    )
                    dst = k_sb if part == 0 else q_sb
                    bh = hp * 2 + b
                    nc.vector.tensor_tensor(
                        dst[:, bh, s0:s0 + QCH], m1[:], m2[:], mybir.AluOpType.add
                    )

            # v: computed directly in k-major [s_tile, 2 heads * Dh],
            # two 128-row tiles per PSUM allocation to halve ACT copies
            for bp in range(QCH // P // 2):
                st0 = s0 // P + 2 * bp
                pv = mmps.tile([P, 2, 256], F32, tag="mm", name="pv")
                for half in range(2):
                    for cci in range(N_CC):
                        nc.tensor.matmul(
                            pv[:, half, :],
                            lhsT=get_xt(cci)[:, (2 * bp + half) * P:(2 * bp + half + 1) * P],
                            rhs=wq_sb[:, 2, cci, :],
                            start=(cci == 0),
                            stop=(cci == N_CC - 1),
                        )
                nc.scalar.activation(
                    v_sb[:, b, st0:st0 + 2, :], pv[:, :, :],
                    mybir.ActivationFunctionType.Copy,
                )

        # ---- Attention + per-batch AllToAll + projection ------------------
        # batch 0: one 1MB AllToAll (fully overlapped by later compute);
        # batch 1: two 512KB AllToAlls (half rows each) so the second one
        # overlaps the first half's projection at the tail
        a2a_in0 = dram.tile([C, ROWS], BF16, name="a2i0")
        a2a_out0 = dram.tile([C, ROWS], BF16, name="a2o0")
        a2a_in1 = [dram.tile([C, ROWS // 2], BF16, name=f"a2i1{h}")
                   for h in range(2)]
        a2a_out1 = [dram.tile([C, ROWS // 2], BF16, name=f"a2o1{h}")
                    for h in range(2)]

        def attn_pair(b, qc):
            """Emit both head-groups of (b, qc) interleaved, with the two
            groups' score tiles paired in one 2-bank PSUM tile so a single
            ACT exp serves each wave (ACT is the attention bottleneck)."""
            n_kt = (QCH // P) * (qc + 1)
            pos = [ops_pool.tile([P, QCH], F32, tag="po", name=f"po{g}")
                   for g in range(2)]
            accs = [acc_pool.tile([P, QCH], BF16, tag="acc", name=f"ac{g}")
                    for g in range(2)]
            pending = {}

            def emit_wave(kt):
                jj = kt - (QCH // P) * qc
                off = P * jj if jj > 0 else 0
                ps2 = sps_pool.tile([P, 2, QCH], F32, tag="sc")
                for hl in range(2):
                    bh = hl * 2 + b
                    nc.tensor.matmul(
                        ps2[:, hl, off:],
                        lhsT=k_sb[:, bh, kt * P:(kt + 1) * P],
                        rhs=q_sb[:, bh, qc * QCH + off:(qc + 1) * QCH],
                        start=True, stop=True,
                    )
                pr2 = probs_pool.tile([P, 2, QCH], BF16, tag="pr")
                nc.scalar.activation(
                    pr2[:, :, off:], ps2[:, :, off:],
                    mybir.ActivationFunctionType.Exp,
                    scale=SCALE,
                )
                for hl in range(2):
                    if jj >= 0:
                        # only the 128-wide diagonal strip needs masking
                        nc.vector.tensor_tensor(
                            pr2[:, hl, off:off + P], pr2[:, hl, off:off + P],
                            tri_sb[:], mybir.AluOpType.mult,
                        )
                    acc = accs[hl]
                    if kt == 0:
                        nc.vector.tensor_copy(acc[:], pr2[:, hl, :])
                    else:
                        nc.vector.tensor_tensor(
                            acc[:, off:], acc[:, off:], pr2[:, hl, off:],
                            mybir.AluOpType.add,
                        )
                pending[kt] = (pr2, off)

            def emit_av(kt):
                pr2, off = pending.pop(kt)
                for hl in range(2):
                    nc.tensor.matmul(
                        pos[hl][:, off:],
                        lhsT=v_sb[:, b, kt, hl * Dh:(hl + 1) * Dh],
                        rhs=pr2[:, hl, off:],
                        start=(kt == 0), stop=(kt == n_kt - 1),
                    )

            for kt in range(n_kt):
                emit_wave(kt)
                if kt >= AV_LAG:
                    emit_av(kt - AV_LAG)
            for kt in range(max(0, n_kt - AV_LAG), n_kt):
                emit_av(kt)

            pd2 = sps_pool.tile([P, 2, QCH], F32, tag="sc", name="pd2")
            for hl in range(2):
                nc.tensor.matmul(
                    pd2[:, hl, :], lhsT=ones_sb[:], rhs=accs[hl][:],
                    start=True, stop=True,
                )
            recip2 = attn_pool.tile([P, 2, QCH], BF16, tag="at", name="recip2")
            with nc.allow_low_precision(reason="softmax denom reciprocal in bf16"):
                nc.vector.reciprocal(recip2[:], pd2[:])
            for hl in range(2):
                at = attn_pool.tile([P, QCH], BF16, tag="at")
                nc.vector.tensor_tensor(
                    at[:], pos[hl][:], recip2[:, hl, :], mybir.AluOpType.mult
                )
                # scatter into the AllToAll input: row-block shards
                for half in range(2):
                    j = 2 * qc + half
                    base = ROWS * j + hl * P
                    if b == 0:
                        nc.sync.dma_start(
                            a2a_in0[base:base + P, :],
                            at[:, half * ROWS:(half + 1) * ROWS],
                        )
                    else:
                        for rh in range(2):
                            c0 = half * ROWS + rh * (ROWS // 2)
                            nc.sync.dma_start(
                                a2a_in1[rh][base:base + P, :],
                                at[:, c0:c0 + ROWS // 2],
                            )

        def alltoall(ain, aout):
            nc.gpsimd.collective_compute(
                "AllToAll",
                mybir.AluOpType.bypass,
                replica_groups=[list(range(N_CORES))],
                ins=[ain[:].opt()],
                outs=[aout[:].opt()],
            )

        def gt_load(aout, pool, tag, rows, name):
            gt = pool.tile([P, N_CC, rows], BF16, tag=tag, name=name)
            nc.gpsimd.dma_start(
                gt[:], aout[:].rearrange("(o p) q -> p o q", p=P)
            )
            return gt

        def proj_rows(gt, sb_list, row0):
            """Project row blocks: for each 128-row block, 4x 512-wide output
            chunks accumulated over 16 contraction slices."""
            for i, sb in enumerate(sb_list):
                for cp in range(2):
                    pjs = [
                        mmps.tile([P, QCH], F32, tag="mm", name=f"pj{h}")
                        for h in range(2)
                    ]
                    for cci in range(N_CC):
                        for half in range(2):
                            co0 = (2 * cp + half) * QCH
                            nc.tensor.matmul(
                                pjs[half][:],
                                lhsT=gt[:, cci, sb * P:(sb + 1) * P],
                                rhs=wproj_sb[:, cci, co0:co0 + QCH],
                                start=(cci == 0),
                                stop=(cci == N_CC - 1),
                            )
                    for half in range(2):
                        ot = outp_pool.tile([P, QCH], F32, tag="ot")
                        nc.vector.tensor_copy(ot[:], pjs[half][:])
                        nc.scalar.dma_start(
                            out_ext[row0 + i * P:row0 + (i + 1) * P,
                                    (2 * cp + half) * QCH:(2 * cp + half + 1) * QCH],
                            ot[:],
                        )

        # ---- Main schedule: QKV with attention pairs interleaved ----------
        for sc in range(N_SC):
            if sc + 1 < N_SC:
                xts[sc + 1] = load_xt(sc + 1)
            qkv_chunk(sc, xts[sc])
            xts[sc] = None
            if sc == 2:
                # off the startup critical path; vector queue reaches this
                # dispatch only after s-chunk 2's rope work
                nc.scalar.dma_start(wproj_sb[:], wproj_in[:])
            if sc >= 1:
                b, qc = divmod(sc - 1, N_QC)
                attn_pair(b, qc)
                if (b, qc) == (0, N_QC - 1):
                    alltoall(a2a_in0, a2a_out0)
                    gt0 = gt_load(a2a_out0, gt_pool, "gt", ROWS, "gt0")
        attn_pair(1, N_QC - 1)
        alltoall(a2a_in1[0], a2a_out1[0])
        gt1h0 = gt_load(a2a_out1[0], xt_pool, "xt", ROWS // 2, "gt1h0")
        alltoall(a2a_in1[1], a2a_out1[1])
        gt1h1 = gt_load(a2a_out1[1], xt_pool, "xt", ROWS // 2, "gt1h1")
        # batch 0's projection fills the PE while batch 1's AllToAlls run
        proj_rows(gt0, [0, 1], 0)
        proj_rows(gt1h0, [0], ROWS)
        proj_rows(gt1h1, [0], ROWS + P)

    nc.finalize()
    return nc


_NC_CACHE = None


def _get_nc():
    global _NC_CACHE
    if _NC_CACHE is None:
        _NC_CACHE = build_nc()
    return _NC_CACHE


def make_in_maps(x, Wqkv, Wproj):
    """Shard + pre-cast + pre-transpose the full inputs on the host."""
    x2 = np.asarray(x, dtype=np.float32).reshape(BS, C).astype(ml_dtypes.bfloat16)
    # xT[p, sc, o, s'] = x2[sc*512 + s', o*128 + p]
    xT = np.ascontiguousarray(
        x2.reshape(N_SC, QCH, N_CC, P).transpose(3, 0, 2, 1)
    )
    Wqkv = np.asarray(Wqkv, dtype=np.float32)
    Wproj = np.asarray(Wproj, dtype=np.float32).astype(ml_dtypes.bfloat16)
    # wproj[p, o, co] = Wproj[o*128 + p, co]
    wproj_a = np.ascontiguousarray(Wproj.reshape(N_CC, P, C).transpose(1, 0, 2))
    cos_t, sin_t, tri, ones = _host_constants()
    in_maps = []
    for i in range(N_CORES):
        h0 = H_LOC * i
        cols = []
        for part in range(3):  # k, q, v blocks (k first per reference)
            base = part * C + h0 * Dh
            cols.append(Wqkv[:, base:base + H_LOC * Dh])
        # [C, 3, 256] -> [p, part, o, col]
        wloc = np.stack(cols, axis=1).astype(ml_dtypes.bfloat16)
        wqkv_a = np.ascontiguousarray(
            wloc.reshape(N_CC, P, 3, 256).transpose(1, 2, 0, 3)
        )
        in_maps.append({
            "xT": xT,
            "wqkv": wqkv_a,
            "wproj": wproj_a,
            "cos_t": cos_t,
            "sin_t": sin_t,
            "tri": tri,
            "ones": ones,
        })
    return in_maps


def assemble_output(results):
    out = np.empty((B, S, C), dtype=np.float32)
    for i in range(N_CORES):
        o = results[i]["out"]                      # [2*ROWS, C]
        for b in range(B):
            out[b, ROWS * i:ROWS * (i + 1), :] = o[b * ROWS:(b + 1) * ROWS, :]
    return out


def kernel(x, Wqkv, Wproj):
    nc = _get_nc()
    in_maps = make_in_maps(x, Wqkv, Wproj)
    res = run_bass_kernel_spmd(nc, in_maps, core_ids=list(range(N_CORES)))
    return assemble_output(res.results)


# revision 34
# speedup vs baseline: 1.3033x; 1.3033x over previous
"""Distributed FlashRotarySelfAttention kernel for 8 TRN2 NeuronCores.

Reference computation (per nn_FlashRotarySelfAttention):
  qkv = x @ Wqkv;  k, q, v = split(qkv, 3)  [k first!]
  k, q = rope(k), rope(q)
  out = causal_softmax(q k^T / sqrt(Dh)) @ v
  return out @ Wproj

Sharding: tensor-parallel over heads for QKV+attention, row-block
parallel for the projection. Core i owns heads {2i, 2i+1}:
  - column-parallel Wqkv (k|q|v columns of its 2 heads)
  - attention fully local per (batch, head); the two head-groups of a
    (batch, q-chunk) pair are emitted interleaved, and the pairs are
    interleaved into the QKV s-chunk loop, so ACT/DVE softmax work
    hides under the PE-bound QKV matmuls and the PE stays warm
  - one small AllToAll per batch redistributes the attention output
    from head-sharded to row-block-sharded (256 rows/core/batch);
    batch 0's AllToAll overlaps batch 1's attention, and batch 0's
    projection matmuls are injected as fillers between the last
    attention pair's matmuls
  - each core then computes the FULL projection (all 2048 output
    channels) for its own 512 rows -> output is row-sharded, no
    further communication

All tensors are pre-cast to bf16 on the host; x is also pre-transposed
on the host, so the kernel does no on-chip casts or transposes.
Matmuls run in bf16 with fp32 PSUM accumulation. RoPE consumes one
ACT copy (PSUM fp32 -> SBUF bf16) + 4 DVE multiplies using a
sign-folded sin table. Softmax skips max-subtraction (scores are O(10)
here); the denominator is accumulated on DVE and reduced across
partitions by a ones-matmul, reciprocal on DVE.
"""

from contextlib import ExitStack

import numpy as np
import ml_dtypes

import concourse.bacc as bacc
import concourse.mybir as mybir
import concourse.tile as tile
from concourse.bass_utils import run_bass_kernel_spmd

# Problem shapes (hardcoded per contest rules).
B, S, C, H = 2, 2048, 2048, 16
Dh = C // H                      # 128
BS = B * S                       # 4096
N_CORES = 8
H_LOC = H // N_CORES             # 2 heads per core
ROWS = S // N_CORES              # 256 output rows per core per batch
ROPE_THETA = 10000.0
SCALE = float(Dh) ** -0.5

F32 = mybir.dt.float32
BF16 = mybir.dt.bfloat16

P = 128            # partitions
QCH = 512          # q-chunk (matmul free dim)
N_SC = BS // QCH   # 8 s-chunks over B*S
N_CC = C // P      # 16 contraction chunks
N_QC = S // QCH    # 4 q-chunks per batch
N_KT = S // P      # 16 k-tiles per batch
AV_LAG = 2         # per-group av lag; x2 effective depth with pair interleave


def _host_constants():
    """Input-independent tables computed on host (compile-time constants)."""
    half = Dh // 2
    inv_freq = 1.0 / (ROPE_THETA ** (np.arange(0, half, dtype=np.float64) / half))
    ang = np.arange(S, dtype=np.float64)[None, :] * inv_freq[:, None]   # [64, S]
    cos_t = np.tile(np.cos(ang), (2, 1)).astype(ml_dtypes.bfloat16)     # [128, S]
    # sign-folded sin, laid out so each RoPE multiply reads both inputs at
    # the same base partition: rows 0-63 (+sin) pair with tb[0:64] to make
    # out[64:128]; rows 64-127 (-sin) pair with tb[64:128] to make out[0:64]
    sin_t = np.concatenate([np.sin(ang), -np.sin(ang)], axis=0).astype(
        ml_dtypes.bfloat16
    )                                                                   # [128, S]
    # upper-triangular (incl diag) strip mask: keep iff q_local >= k_local
    kk = np.arange(P)[:, None]
    cc = np.arange(P)[None, :]
    tri = (cc >= kk).astype(ml_dtypes.bfloat16)                         # [128, 128]
    ones = np.ones((P, P), dtype=ml_dtypes.bfloat16)
    return cos_t, sin_t, tri, ones


def build_nc():
    nc = bacc.Bacc(None, num_devices=N_CORES)

    xt_in = nc.declare_dram_parameter("xT", [P, N_SC, N_CC, QCH], BF16, isOutput=False)
    wqkv_in = nc.declare_dram_parameter("wqkv", [P, 3, N_CC, 256], BF16, isOutput=False)
    wproj_in = nc.declare_dram_parameter("wproj", [P, N_CC, C], BF16, isOutput=False)
    cos_in = nc.declare_dram_parameter("cos_t", [P, S], BF16, isOutput=False)
    sin_in = nc.declare_dram_parameter("sin_t", [P, S], BF16, isOutput=False)
    tri_in = nc.declare_dram_parameter("tri", [P, P], BF16, isOutput=False)
    ones_in = nc.declare_dram_parameter("ones", [P, P], BF16, isOutput=False)
    out_ext = nc.declare_dram_parameter("out", [B * ROWS, C], F32, isOutput=True)

    with tile.TileContext(nc) as tc, ExitStack() as ctx:
        consts = ctx.enter_context(tc.tile_pool(name="consts", bufs=1))
        qkvp = ctx.enter_context(tc.tile_pool(name="qkvp", bufs=1))
        xt_pool = ctx.enter_context(tc.tile_pool(name="xt", bufs=2))
        rope_pool = ctx.enter_context(tc.tile_pool(name="rope", bufs=3))
        probs_pool = ctx.enter_context(tc.tile_pool(name="probs", bufs=4))
        acc_pool = ctx.enter_context(tc.tile_pool(name="accs", bufs=2))
        attn_pool = ctx.enter_context(tc.tile_pool(name="attn", bufs=3))
        gt_pool = ctx.enter_context(tc.tile_pool(name="gt", bufs=1))
        outp_pool = ctx.enter_context(tc.tile_pool(name="outp", bufs=2))
        dram = ctx.enter_context(tc.tile_pool(name="dram", bufs=1, space="DRAM"))
        mmps = ctx.enter_context(tc.tile_pool(name="mmps", bufs=2, space="PSUM"))
        sps_pool = ctx.enter_context(tc.tile_pool(name="sps", bufs=2, space="PSUM"))
        ops_pool = ctx.enter_context(tc.tile_pool(name="ops", bufs=2, space="PSUM"))

        # ---- Startup DMAs: k weights + first x half first -----------------
        wq_sb = consts.tile([P, 3, N_CC, 256], BF16)
        nc.gpsimd.dma_start(wq_sb[:, 0, :, :], wqkv_in[:, 0, :, :])

        xts = [None] * N_SC

        def load_xt(sc):
            xt = xt_pool.tile([P, N_CC, QCH], BF16, tag="xt", name=f"xt{sc}")
            nc.sync.dma_start(xt[:], xt_in[:, sc, :, :])
            return lambda cc: xt[:, cc, :]

        # chunk 0 in two halves so the first k-matmul chain can start after
        # just 1 MB of x has landed
        x0a = xt_pool.tile([P, N_CC // 2, QCH], BF16, tag="xt", name="x0a")
        nc.sync.dma_start(x0a[:], xt_in[:, 0, 0:N_CC // 2, :])
        x0b = xt_pool.tile([P, N_CC // 2, QCH], BF16, tag="xt", name="x0b")
        nc.sync.dma_start(x0b[:], xt_in[:, 0, N_CC // 2:, :])
        xts[0] = lambda cc: (x0a if cc < N_CC // 2 else x0b)[:, cc % (N_CC // 2), :]

        nc.gpsimd.dma_start(wq_sb[:, 1, :, :], wqkv_in[:, 1, :, :])
        nc.gpsimd.dma_start(wq_sb[:, 2, :, :], wqkv_in[:, 2, :, :])

        cos_sb = consts.tile([P, S], BF16)
        nc.scalar.dma_start(cos_sb[:], cos_in[:])
        sin_sb = consts.tile([P, S], BF16)
        nc.scalar.dma_start(sin_sb[:], sin_in[:])
        tri_sb = consts.tile([P, P], BF16)
        nc.scalar.dma_start(tri_sb[:], tri_in[:])
        ones_sb = consts.tile([P, P], BF16)
        nc.scalar.dma_start(ones_sb[:], ones_in[:])

        # wproj is loaded later (emitted after s-chunk 2) off the critical
        # startup HBM window
        wproj_sb = consts.tile([P, N_CC, C], BF16)

        # Resident activations: d-major q/k, k-major v. bh = hl*2 + b
        q_sb = qkvp.tile([P, 2 * H_LOC, S], BF16)
        k_sb = qkvp.tile([P, 2 * H_LOC, S], BF16)
        v_sb = qkvp.tile([P, B, N_KT, H_LOC * Dh], BF16)

        def qkv_chunk(sc, get_xt):
            b, s0 = divmod(sc, N_QC)
            s0 *= QCH                      # position offset within batch
            # k (part 0) and q (part 1): matmul -> RoPE -> bf16 resident
            for part in range(2):
                for hp in range(H_LOC):
                    ps = mmps.tile([P, QCH], F32, tag="mm")
                    for cci in range(N_CC):
                        nc.tensor.matmul(
                            ps[:],
                            lhsT=wq_sb[:, part, cci, hp * P:(hp + 1) * P],
                            rhs=get_xt(cci),
                            start=(cci == 0),
                            stop=(cci == N_CC - 1),
                        )
                    tb = rope_pool.tile([P, QCH], BF16, tag="rt")
                    nc.scalar.activation(
                        tb[:], ps[:], mybir.ActivationFunctionType.Copy
                    )
                    m1 = rope_pool.tile([P, QCH], BF16, tag="rt")
                    m2 = rope_pool.tile([P, QCH], BF16, tag="rt")
                    nc.vector.tensor_tensor(
                        m1[:], tb[:], cos_sb[:, s0:s0 + QCH], mybir.AluOpType.mult
                    )
                    nc.vector.tensor_tensor(
                        m2[0:64, :], tb[64:128, :], sin_sb[64:128, s0:s0 + QCH],
                        mybir.AluOpType.mult,
                    )
                    nc.vector.tensor_tensor(
                        m2[64:128, :], tb[0:64, :], sin_sb[0:64, s0:s0 + QCH],
                        mybir.AluOpType.mult,
                    )
                    dst = k_sb if part == 0 else q_sb
                    bh = hp * 2 + b
                    nc.vector.tensor_tensor(
                        dst[:, bh, s0:s0 + QCH], m1[:], m2[:], mybir.AluOpType.add
                    )

            # v: computed directly in k-major [s_tile, 2 heads * Dh],
            # two 128-row tiles per PSUM allocation to halve ACT copies
            for bp in range(QCH // P // 2):
                st0 = s0 // P + 2 * bp
                pv = mmps.tile([P, 2, 256], F32, tag="mm", name="pv")
                for half in range(2):
                    for cci in range(N_CC):
                        nc.tensor.matmul(
                            pv[:, half, :],
                            lhsT=get_xt(cci)[:, (2 * bp + half) * P:(2 * bp + half + 1) * P],
                            rhs=wq_sb[:, 2, cci, :],
                            start=(cci == 0),
                            stop=(cci == N_CC - 1),
                        )
                nc.vector.tensor_copy(v_sb[:, b, st0:st0 + 2, :], pv[:, :, :])

        # ---- Attention + per-batch AllToAll + projection ------------------
        # batch 0: one 1MB AllToAll (fully overlapped by later compute);
        # batch 1: two 512KB AllToAlls (half rows each) so the second one
        # overlaps the first half's projection at the tail
        a2a_in0 = dram.tile([C, ROWS], BF16, name="a2i0")
        a2a_out0 = dram.tile([C, ROWS], BF16, name="a2o0")
        a2a_in1 = [dram.tile([C, ROWS // 2], BF16, name=f"a2i1{h}")
                   for h in range(2)]
        a2a_out1 = [dram.tile([C, ROWS // 2], BF16, name=f"a2o1{h}")
                    for h in range(2)]

        def attn_pair(b, qc):
            """Emit both head-groups of (b, qc) interleaved, with the two
            groups' score tiles paired in one 2-bank PSUM tile so a single
            ACT exp serves each wave (ACT is the attention bottleneck)."""
            n_kt = (QCH // P) * (qc + 1)
            pos = [ops_pool.tile([P, QCH], F32, tag="po", name=f"po{g}")
                   for g in range(2)]
            accs = [acc_pool.tile([P, QCH], BF16, tag="acc", name=f"ac{g}")
                    for g in range(2)]
            pending = {}

            def emit_wave(kt):
                jj = kt - (QCH // P) * qc
                off = P * jj if jj > 0 else 0
                ps2 = sps_pool.tile([P, 2, QCH], F32, tag="sc")
                for hl in range(2):
                    bh = hl * 2 + b
                    nc.tensor.matmul(
                        ps2[:, hl, off:],
                        lhsT=k_sb[:, bh, kt * P:(kt + 1) * P],
                        rhs=q_sb[:, bh, qc * QCH + off:(qc + 1) * QCH],
                        start=True, stop=True,
                    )
                pr2 = probs_pool.tile([P, 2, QCH], BF16, tag="pr")
                nc.scalar.activation(
                    pr2[:, :, off:], ps2[:, :, off:],
                    mybir.ActivationFunctionType.Exp,
                    scale=SCALE,
                )
                for hl in range(2):
                    if jj >= 0:
                        # only the 128-wide diagonal strip needs masking
                        nc.vector.tensor_tensor(
                            pr2[:, hl, off:off + P], pr2[:, hl, off:off + P],
                            tri_sb[:], mybir.AluOpType.mult,
                        )
                    acc = accs[hl]
                    if kt == 0:
                        nc.vector.tensor_copy(acc[:], pr2[:, hl, :])
                    else:
                        nc.vector.tensor_tensor(
                            acc[:, off:], acc[:, off:], pr2[:, hl, off:],
                            mybir.AluOpType.add,
                        )
                pending[kt] = (pr2, off)

            def emit_av(kt):
                pr2, off = pending.pop(kt)
                for hl in range(2):
                    nc.tensor.matmul(
                        pos[hl][:, off:],
                        lhsT=v_sb[:, b, kt, hl * Dh:(hl + 1) * Dh],
                        rhs=pr2[:, hl, off:],
                        start=(kt == 0), stop=(kt == n_kt - 1),
                    )

            for kt in range(n_kt):
                emit_wave(kt)
                if kt >= AV_LAG:
                    emit_av(kt - AV_LAG)
            for kt in range(max(0, n_kt - AV_LAG), n_kt):
                emit_av(kt)

            pd2 = sps_pool.tile([P, 2, QCH], F32, tag="sc", name="pd2")
            for hl in range(2):
                nc.tensor.matmul(
                    pd2[:, hl, :], lhsT=ones_sb[:], rhs=accs[hl][:],
                    start=True, stop=True,
                )
            recip2 = attn_pool.tile([P, 2, QCH], BF16, tag="at", name="recip2")
            with nc.allow_low_precision(reason="softmax denom reciprocal in bf16"):
                nc.vector.reciprocal(recip2[:], pd2[:])
            for hl in range(2):
                at = attn_pool.tile([P, QCH], BF16, tag="at")
                nc.vector.tensor_tensor(
                    at[:], pos[hl][:], recip2[:, hl, :], mybir.AluOpType.mult
                )
                # scatter into the AllToAll input: row-block shards
                for half in range(2):
                    j = 2 * qc + half
                    base = ROWS * j + hl * P
                    if b == 0:
                        nc.sync.dma_start(
                            a2a_in0[base:base + P, :],
                            at[:, half * ROWS:(half + 1) * ROWS],
                        )
                    else:
                        for rh in range(2):
                            c0 = half * ROWS + rh * (ROWS // 2)
                            nc.sync.dma_start(
                                a2a_in1[rh][base:base + P, :],
                                at[:, c0:c0 + ROWS // 2],
                            )

        def alltoall(ain, aout):
            nc.gpsimd.collective_compute(
                "AllToAll",
                mybir.AluOpType.bypass,
                replica_groups=[list(range(N_CORES))],
                ins=[ain[:].opt()],
                outs=[aout[:].opt()],
            )

        def gt_load(aout, pool, tag, rows, name):
            gt = pool.tile([P, N_CC, rows], BF16, tag=tag, name=name)
            nc.gpsimd.dma_start(
                gt[:], aout[:].rearrange("(o p) q -> p o q", p=P)
            )
            return gt

        def proj_rows(gt, sb_list, row0):
            """Project row blocks: for each 128-row block, 4x 512-wide output
            chunks accumulated over 16 contraction slices."""
            for i, sb in enumerate(sb_list):
                for cp in range(2):
                    pjs = [
                        mmps.tile([P, QCH], F32, tag="mm", name=f"pj{h}")
                        for h in range(2)
                    ]
                    for cci in range(N_CC):
                        for half in range(2):
                            co0 = (2 * cp + half) * QCH
                            nc.tensor.matmul(
                                pjs[half][:],
                                lhsT=gt[:, cci, sb * P:(sb + 1) * P],
                                rhs=wproj_sb[:, cci, co0:co0 + QCH],
                                start=(cci == 0),
                                stop=(cci == N_CC - 1),
                            )
                    for half in range(2):
                        ot = outp_pool.tile([P, QCH], F32, tag="ot")
                        nc.vector.tensor_copy(ot[:], pjs[half][:])
                        nc.scalar.dma_start(
                            out_ext[row0 + i * P:row0 + (i + 1) * P,
                                    (2 * cp + half) * QCH:(2 * cp + half + 1) * QCH],
                            ot[:],
                        )

        # ---- Main schedule: QKV with attention pairs interleaved ----------
        for sc in range(N_SC):
            if sc + 1 < N_SC:
                xts[sc + 1] = load_xt(sc + 1)
            qkv_chunk(sc, xts[sc])
            xts[sc] = None
            if sc == 2:
                # off the startup critical path; vector queue reaches this
                # dispatch only after s-chunk 2's rope work
                nc.scalar.dma_start(wproj_sb[:], wproj_in[:])
            if sc >= 1:
                b, qc = divmod(sc - 1, N_QC)
                attn_pair(b, qc)
                if (b, qc) == (0, N_QC - 1):
                    alltoall(a2a_in0, a2a_out0)
                    gt0 = gt_load(a2a_out0, gt_pool, "gt", ROWS, "gt0")
        attn_pair(1, N_QC - 1)
        alltoall(a2a_in1[0], a2a_out1[0])
        gt1h0 = gt_load(a2a_out1[0], xt_pool, "xt", ROWS // 2, "gt1h0")
        alltoall(a2a_in1[1], a2a_out1[1])
        gt1h1 = gt_load(a2a_out1[1], xt_pool, "xt", ROWS // 2, "gt1h1")
        # batch 0's projection fills the PE while batch 1's AllToAlls run
        proj_rows(gt0, [0, 1], 0)
        proj_rows(gt1h0, [0], ROWS)
        proj_rows(gt1h1, [0], ROWS + P)

    nc.finalize()
    return nc


_NC_CACHE = None


def _get_nc():
    global _NC_CACHE
    if _NC_CACHE is None:
        _NC_CACHE = build_nc()
    return _NC_CACHE


def make_in_maps(x, Wqkv, Wproj):
    """Shard + pre-cast + pre-transpose the full inputs on the host."""
    x2 = np.asarray(x, dtype=np.float32).reshape(BS, C).astype(ml_dtypes.bfloat16)
    # xT[p, sc, o, s'] = x2[sc*512 + s', o*128 + p]
    xT = np.ascontiguousarray(
        x2.reshape(N_SC, QCH, N_CC, P).transpose(3, 0, 2, 1)
    )
    Wqkv = np.asarray(Wqkv, dtype=np.float32)
    Wproj = np.asarray(Wproj, dtype=np.float32).astype(ml_dtypes.bfloat16)
    # wproj[p, o, co] = Wproj[o*128 + p, co]
    wproj_a = np.ascontiguousarray(Wproj.reshape(N_CC, P, C).transpose(1, 0, 2))
    cos_t, sin_t, tri, ones = _host_constants()
    in_maps = []
    for i in range(N_CORES):
        h0 = H_LOC * i
        cols = []
        for part in range(3):  # k, q, v blocks (k first per reference)
            base = part * C + h0 * Dh
            cols.append(Wqkv[:, base:base + H_LOC * Dh])
        # [C, 3, 256] -> [p, part, o, col]
        wloc = np.stack(cols, axis=1).astype(ml_dtypes.bfloat16)
        wqkv_a = np.ascontiguousarray(
            wloc.reshape(N_CC, P, 3, 256).transpose(1, 2, 0, 3)
        )
        in_maps.append({
            "xT": xT,
            "wqkv": wqkv_a,
            "wproj": wproj_a,
            "cos_t": cos_t,
            "sin_t": sin_t,
            "tri": tri,
            "ones": ones,
        })
    return in_maps


def assemble_output(results):
    out = np.empty((B, S, C), dtype=np.float32)
    for i in range(N_CORES):
        o = results[i]["out"]                      # [2*ROWS, C]
        for b in range(B):
            out[b, ROWS * i:ROWS * (i + 1), :] = o[b * ROWS:(b + 1) * ROWS, :]
    return out


def kernel(x, Wqkv, Wproj):
    nc = _get_nc()
    in_maps = make_in_maps(x, Wqkv, Wproj)
    res = run_bass_kernel_spmd(nc, in_maps, core_ids=list(range(N_CORES)))
    return assemble_output(res.results)
